# revision 11
# baseline (speedup 1.0000x reference)
"""DynamicMemoryRouter TRN2 Bass kernel (v3, token-split).

Sharding: 8 cores = B(4) x token-half(2). Core i owns batch b=i//2 and
token half g=i%2 (2048 tokens), computing ALL 16 heads for its tokens.
The softmax over the query dim N is the only cross-core coupling: each
core computes partial Z[h,s] (sum of exp over its token half) and a tiny
per-head AllReduce (pairwise, [128,4] f32) completes it. No og exchange,
no ReduceScatter of activations.

Per-core pipeline:
  LN1 (chunked by 512-token groups): stats via ones-matmuls on x^T
  tiles, per-chunk packed finalize (DMA round trip) -> rstd and
  q = mu*rstd broadcasts, normalize in place: xg = x*rb - qb.
  ln_g is folded into Mk host-side, ln_b into a per-slot exp bias.

  Attention per head h: scores = Mkg^T x (f32r, PSUM 1024-col chunks)
  -> exp with accumulated partial Z -> DMA Z to dram -> pairwise
  AllReduce(add) -> invZ -> fold invZ into Mv stationary (bf16, with a
  ones column producing the slot-renorm denom D) -> O accumulation
  ([65,512] PSUM at base 0) -> copy O rows into packed og tiles (two
  heads per 128-partition tile; the odd head's 64-partition DVE copy
  writes quadrants 2/3 while reading 0/1) -> D packed-reciprocal ->
  per-pair renorm og *= db in place.

  conv: C = Wo^T @ og (og SBUF-resident, full D contraction); y = x + C
  with x re-streamed from HBM; LN2 stats ones-matmuls folded per token
  chunk with chunked packed finalize (ln2_g/b folded into W1/b1).

  FFN per 512-token chunk: h0c = y*rb2 - qb2 (bf16); W1 matmul bf16
  (SBUF-resident); GELU -> g1 fp8e4 in [128,2,512] DoubleRow layout;
  W2 matmul fp8e4 perf_mode=DoubleRow (W2 host-quantized *2048,
  streamed); epilogue pms/2048 + b2 + y -> yout.
"""

import os
import sys

for _p in ("/opt/trn_rl_repo", "/root/.axon_site/_ro/trn_rl_repo"):
    if os.path.isdir(_p) and _p not in sys.path:
        sys.path.insert(0, _p)

import numpy as np
import ml_dtypes

import concourse.bass as bass
import concourse.tile as tile
from concourse import bacc, mybir
from concourse.bass_utils import run_bass_kernel_spmd

F32 = mybir.dt.float32
F32R = mybir.dt.float32r
BF16 = mybir.dt.bfloat16
F8E4 = mybir.dt.float8e4
AF = mybir.ActivationFunctionType
ALU = mybir.AluOpType
DR = mybir.MatmulPerfMode.DoubleRow

B, N, D = 4, 4096, 1024
H, S = 16, 512
DH = D // H
DFF = 4 * D
P = 128
NC = 512
NT = N // 2          # tokens owned per core
LN_EPS = 1e-5
SLOT_EPS = 1e-9
W2_SCALE = 2048.0

_CACHED = {}


def _bcast_ap(dram_tile, row_offset_elems, width, parts):
    return bass.AP(
        tensor=dram_tile.tensor,
        offset=dram_tile.offset + row_offset_elems,
        ap=[[0, parts], [1, width]],
    )


def _emit_ln1(nc, tc, io, dr, cst, xg):
    """Chunked LN1: per 512-token chunk compute stats (ones-matmuls),
    packed finalize, and normalize xg in place: xg = x*rb - qb."""
    with (
        tc.tile_pool(name="sqp", bufs=2) as sqp,
        tc.tile_pool(name="rows1", bufs=4) as rows,
        tc.tile_pool(name="pk1", bufs=2) as pkp,
        tc.tile_pool(name="bc1", bufs=2) as bcp,
        tc.tile_pool(name="tmp1", bufs=4) as tmpp,
        tc.tile_pool(name="ps_st", bufs=4, space="PSUM") as ps_st,
    ):
        for nch in range(4):
            c0 = nch * NC
            ps_sum = ps_st.tile([1, NC], F32, tag="ps_st", name="ps_st")
            ps_sq = ps_st.tile([1, NC], F32, tag="ps_st", name="ps_st")
            for dt in range(8):
                ch = xg[dt][:, c0:c0 + NC]
                nc.sync.dma_start(
                    out=ch, in_=io.xt[dt * P:(dt + 1) * P, c0:c0 + NC]
                )
                sq = sqp.tile([P, NC], F32R, tag="sq", name="sq")
                nc.scalar.square(sq[:, :], ch)
                nc.tensor.matmul(
                    ps_sum[:, :], cst.onesr[:, :], ch,
                    start=(dt == 0), stop=(dt == 7),
                )
                nc.tensor.matmul(
                    ps_sq[:, :], cst.onesr[:, :], sq[:, :],
                    start=(dt == 0), stop=(dt == 7),
                )
            r_sum = rows.tile([1, NC], F32, tag="r1", name="r1")
            r_sq = rows.tile([1, NC], F32, tag="r1", name="r1")
            nc.scalar.copy(r_sum[:, :], ps_sum[:, :])
            nc.scalar.copy(r_sq[:, :], ps_sq[:, :])
            nc.sync.dma_start(out=dr.s1d[0:1, c0:c0 + NC], in_=r_sum)
            nc.sync.dma_start(out=dr.s1d[1:2, c0:c0 + NC], in_=r_sq)

            # packed finalize for this chunk: [1,512] rows -> [128,4]
            pk = pkp.tile([P, 8], F32, tag="pk", name="pk")
            nc.sync.dma_start(
                out=pk[:, 0:4],
                in_=dr.s1d[0:1, c0:c0 + NC].rearrange("o (p c) -> (o p) c", p=P),
            )
            nc.sync.dma_start(
                out=pk[:, 4:8],
                in_=dr.s1d[1:2, c0:c0 + NC].rearrange("o (p c) -> (o p) c", p=P),
            )
            mean = pkp.tile([P, 4], F32, tag="mean", name="mean")
            nc.vector.tensor_scalar_mul(mean[:, :], pk[:, 0:4], 1.0 / D)
            msq = pkp.tile([P, 4], F32, tag="msq", name="msq")
            nc.vector.tensor_mul(msq[:, :], mean[:, :], mean[:, :])
            var = pkp.tile([P, 4], F32, tag="var", name="var")
            nc.vector.scalar_tensor_tensor(
                out=var[:, :], in0=pk[:, 4:8], scalar=1.0 / D,
                in1=msq[:, :], op0=ALU.mult, op1=ALU.subtract,
            )
            nc.vector.tensor_scalar_add(var[:, :], var[:, :], LN_EPS)
            sd = pkp.tile([P, 4], F32, tag="sd", name="sd")
            nc.scalar.sqrt(sd[:, :], var[:, :])
            rstd = pkp.tile([P, 4], F32, tag="rstd", name="rstd")
            nc.vector.reciprocal_approx_fast(out=rstd[:, :], in_=sd[:, :])
            q = pkp.tile([P, 4], F32, tag="q", name="q")
            nc.vector.tensor_mul(q[:, :], mean[:, :], rstd[:, :])
            nc.sync.dma_start(
                out=dr.r1d[0:1, c0:c0 + NC].rearrange("o (p c) -> (o p) c", p=P),
                in_=q,
            )
            nc.sync.dma_start(
                out=dr.r1d[1:2, c0:c0 + NC].rearrange("o (p c) -> (o p) c", p=P),
                in_=rstd,
            )
            qb = bcp.tile([P, NC], F32, tag="qb", name="qb")
            rb = bcp.tile([P, NC], F32, tag="rb", name="rb")
            nc.sync.dma_start(out=qb, in_=_bcast_ap(dr.r1d, c0, NC, P))
            nc.sync.dma_start(out=rb, in_=_bcast_ap(dr.r1d, NT + c0, NC, P))

            for dt in range(8):
                tmp = tmpp.tile([P, NC], F32, tag="tmp", name="tmp")
                nc.vector.tensor_mul(tmp[:, :], xg[dt][:, c0:c0 + NC], rb[:, :])
                nc.vector.tensor_sub(xg[dt][:, c0:c0 + NC], tmp[:, :], qb[:, :])


def _emit_attention(nc, tc, io, dr, cst, xg, og, groups, stage):
    """Per head: scores (f32r), exp with partial-Z accum, per-head Z
    AllReduce, invZ-folded O accumulation, og pack + slot renorm."""
    n_heads = int(os.environ.get("KERNEL_HEADS", str(H)))
    with (
        tc.tile_pool(name="epool", bufs=12) as epool,
        tc.tile_pool(name="mvap", bufs=6) as mvap,
        tc.tile_pool(name="mvsp", bufs=6) as mvsp,
        tc.tile_pool(name="zp", bufs=3) as zp,
        tc.tile_pool(name="dbp", bufs=1) as dbp,
        tc.tile_pool(name="pkdp", bufs=3) as pkdp,
        tc.tile_pool(name="ps_sc", bufs=3, space="PSUM") as ps_sc,
        tc.tile_pool(name="ps_o", bufs=2, space="PSUM") as ps_o,
    ):
        def scores_exp(h):
            j, hb = h // 2, (h % 2) * 64
            zc = zp.tile([P, 8], F32, tag="zc", name="zc")
            et = []
            for st in range(4):
                e_st = epool.tile([P, NT], BF16, tag="e", name="e")
                for cc in range(2):
                    ps = ps_sc.tile([P, 1024], F32, tag="ps_sc", name="ps_sc")
                    for k in range(2):
                        t0 = cc * 1024 + k * NC
                        nc.tensor.matmul(
                            ps[:, k * NC:(k + 1) * NC],
                            io.mktp_sb[j][hb:hb + 64, st * P:(st + 1) * P],
                            xg[j][hb:hb + 64, t0:t0 + NC],
                            start=True, stop=True,
                        )
                    nc.scalar.activation(
                        out=e_st[:, cc * 1024:(cc + 1) * 1024], in_=ps[:, :],
                        func=AF.Exp, bias=cst.cb_sb[:, h * 4 + st:h * 4 + st + 1],
                        accum_out=zc[:, st * 2 + cc:st * 2 + cc + 1],
                    )
                et.append(e_st)
            zph = zp.tile([P, 4], F32, tag="zph", name="zph")
            for st in range(4):
                nc.vector.tensor_add(
                    zph[:, st:st + 1], zc[:, 2 * st:2 * st + 1],
                    zc[:, 2 * st + 1:2 * st + 2],
                )
            nc.sync.dma_start(out=dr.zin[h, :, :], in_=zph)
            if stage >= 3:
                nc.gpsimd.collective_compute(
                    "AllReduce", ALU.add, replica_groups=groups,
                    ins=[dr.zin[h, :, :]], outs=[dr.zout[h, :, :]],
                )
            return et

        def o_accum(h, et):
            j, hb = h // 2, (h % 2) * 64
            zall = zp.tile([P, 4], F32, tag="zall", name="zall")
            src = dr.zout if stage >= 3 else dr.zin
            nc.sync.dma_start(out=zall, in_=src[h, :, :])
            invZ = zp.tile([P, 4], F32, tag="invZ", name="invZ")
            nc.vector.reciprocal_approx_fast(out=invZ[:, :], in_=zall[:, :])
            mvs = []
            for st in range(4):
                mva_t = mvap.tile([P, 65], F32, tag="mva", name="mva")
                nc.sync.dma_start(out=mva_t, in_=io.mva[h * 4 + st, :, :])
                mvs_t = mvsp.tile([P, 65], BF16, tag="mvs", name="mvs")
                nc.vector.tensor_scalar_mul(
                    mvs_t[:, :], mva_t[:, :], invZ[:, st:st + 1]
                )
                mvs.append(mvs_t)
            den = zp.tile([1, NT], F32, tag="den", name="den")
            for c in range(4):
                po = ps_o.tile([65, NC], F32, tag="ps_o", name="ps_o")
                for st in range(4):
                    nc.tensor.matmul(
                        po[:, :], mvs[st][:, :],
                        et[st][:, c * NC:(c + 1) * NC],
                        start=(st == 0), stop=(st == 3),
                    )
                nc.vector.tensor_copy(
                    og[j][hb:hb + 64, c * NC:(c + 1) * NC], po[0:64, :]
                )
                nc.vector.tensor_copy(
                    den[0:1, c * NC:(c + 1) * NC], po[64:65, :]
                )
            nc.sync.dma_start(out=dr.dinv_raw[h:h + 1, :], in_=den[0:1, :])
            # packed reciprocal of (eps + D)
            pkd = pkdp.tile([P, 16], F32, tag="pkd", name="pkd")
            nc.sync.dma_start(
                out=pkd,
                in_=dr.dinv_raw[h:h + 1, :].rearrange("o (p c) -> (o p) c", p=P),
            )
            pkf = pkdp.tile([P, 16], F32, tag="pkf", name="pkf")
            nc.vector.tensor_scalar_add(pkf[:, :], pkd[:, :], SLOT_EPS)
            pki = pkdp.tile([P, 16], F32, tag="pki", name="pki")
            nc.vector.reciprocal_approx_fast(out=pki[:, :], in_=pkf[:, :])
            nc.sync.dma_start(
                out=dr.dinv_inv[h:h + 1, :].rearrange("o (p c) -> (o p) c", p=P),
                in_=pki,
            )
            if h % 2 == 1:
                db = dbp.tile([P, NT], F32, tag="db", name="db")
                nc.sync.dma_start(
                    out=db[0:64, :], in_=_bcast_ap(dr.dinv_inv, (h - 1) * NT, NT, 64)
                )
                nc.sync.dma_start(
                    out=db[64:128, :], in_=_bcast_ap(dr.dinv_inv, h * NT, NT, 64)
                )
                nc.vector.tensor_mul(og[j][:, :], og[j][:, :], db[:, :])
                nc.sync.dma_start(
                    out=dr.ogd[j * P:(j + 1) * P, :], in_=og[j][:, :]
                )

        # software pipeline: scores(h+1) overlaps AllReduce(h) / O(h)
        pend = []
        for h in range(n_heads):
            et = scores_exp(h)
            pend.append((h, et))
            if len(pend) >= 2:
                o_accum(*pend.pop(0))
        for h, et in pend:
            o_accum(h, et)


def _emit_conv(nc, tc, io, dr, cst, y, wot_sb):
    """C = Wo^T @ og (og streamed from DRAM); y = x + C (x re-streamed);
    LN2 stats + chunked packed finalize."""
    with (
        tc.tile_pool(name="ogs", bufs=16) as ogsp,
        tc.tile_pool(name="xtr", bufs=4) as xtrp,
        tc.tile_pool(name="sq2p", bufs=2) as sq2p,
        tc.tile_pool(name="rows2", bufs=4) as rows2,
        tc.tile_pool(name="pk2", bufs=2) as pk2p,
        tc.tile_pool(name="ps_c", bufs=3, space="PSUM") as ps_c,
        tc.tile_pool(name="ps_s2", bufs=4, space="PSUM") as ps_s2,
    ):
        for nch in range(4):
            c0 = nch * NC
            mg = [ogsp.tile([P, NC], BF16, tag="mg", name="mg")
                  for _ in range(8)]
            for kc in range(8):
                nc.sync.dma_start(
                    out=mg[kc], in_=dr.ogd[kc * P:(kc + 1) * P, c0:c0 + NC]
                )
            for do in range(8):
                pc = ps_c.tile([P, NC], F32, tag="ps_c", name="ps_c")
                for kc in range(8):
                    nc.tensor.matmul(
                        pc[:, :], wot_sb[kc][:, do * P:(do + 1) * P],
                        mg[kc][:, :], start=(kc == 0), stop=(kc == 7),
                    )
                xtr = xtrp.tile([P, NC], F32, tag="xtr", name="xtr")
                nc.sync.dma_start(
                    out=xtr, in_=io.xt[do * P:(do + 1) * P, c0:c0 + NC].bitcast(F32)
                )
                nc.vector.tensor_add(y[do][:, c0:c0 + NC], pc[:, :], xtr[:, :])
            ps2_sum = ps_s2.tile([1, NC], F32, tag="ps2", name="ps2")
            ps2_sq = ps_s2.tile([1, NC], F32, tag="ps2", name="ps2")
            for dt in range(8):
                ysl = y[dt][:, c0:c0 + NC]
                sq = sq2p.tile([P, NC], F32R, tag="sq2", name="sq2")
                nc.scalar.square(sq[:, :], ysl)
                nc.tensor.matmul(
                    ps2_sum[:, :], cst.onesr[:, :], ysl,
                    start=(dt == 0), stop=(dt == 7),
                )
                nc.tensor.matmul(
                    ps2_sq[:, :], cst.onesr[:, :], sq[:, :],
                    start=(dt == 0), stop=(dt == 7),
                )
            r2a = rows2.tile([1, NC], F32, tag="r2", name="r2")
            r2b = rows2.tile([1, NC], F32, tag="r2", name="r2")
            nc.scalar.copy(r2a[:, :], ps2_sum[:, :])
            nc.scalar.copy(r2b[:, :], ps2_sq[:, :])
            nc.sync.dma_start(out=dr.s2d[0:1, c0:c0 + NC], in_=r2a)
            nc.sync.dma_start(out=dr.s2d[1:2, c0:c0 + NC], in_=r2b)

            pk = pk2p.tile([P, 8], F32, tag="pk2", name="pk2")
            nc.sync.dma_start(
                out=pk[:, 0:4],
                in_=dr.s2d[0:1, c0:c0 + NC].rearrange("o (p c) -> (o p) c", p=P),
            )
            nc.sync.dma_start(
                out=pk[:, 4:8],
                in_=dr.s2d[1:2, c0:c0 + NC].rearrange("o (p c) -> (o p) c", p=P),
            )
            mean = pk2p.tile([P, 4], F32, tag="mean2", name="mean2")
            nc.vector.tensor_scalar_mul(mean[:, :], pk[:, 0:4], 1.0 / D)
            msq = pk2p.tile([P, 4], F32, tag="msq2", name="msq2")
            nc.vector.tensor_mul(msq[:, :], mean[:, :], mean[:, :])
            var = pk2p.tile([P, 4], F32, tag="var2", name="var2")
            nc.vector.scalar_tensor_tensor(
                out=var[:, :], in0=pk[:, 4:8], scalar=1.0 / D,
                in1=msq[:, :], op0=ALU.mult, op1=ALU.subtract,
            )
            nc.vector.tensor_scalar_add(var[:, :], var[:, :], LN_EPS)
            sd = pk2p.tile([P, 4], F32, tag="sd2", name="sd2")
            nc.scalar.sqrt(sd[:, :], var[:, :])
            rstd = pk2p.tile([P, 4], F32, tag="rstd2", name="rstd2")
            nc.vector.reciprocal_approx_fast(out=rstd[:, :], in_=sd[:, :])
            q = pk2p.tile([P, 4], F32, tag="q2", name="q2")
            nc.vector.tensor_mul(q[:, :], mean[:, :], rstd[:, :])
            nc.sync.dma_start(
                out=dr.r2d[0:1, c0:c0 + NC].rearrange("o (p c) -> (o p) c", p=P),
                in_=q,
            )
            nc.sync.dma_start(
                out=dr.r2d[1:2, c0:c0 + NC].rearrange("o (p c) -> (o p) c", p=P),
                in_=rstd,
            )


def _emit_ffn(nc, tc, io, dr, cst, y, w1_sb):
    """Per 512-token chunk: h0c = y*rb2 - qb2 (bf16); W1 bf16 matmul;
    GELU -> g1 fp8 DoubleRow layout; W2 fp8 DoubleRow (streamed);
    epilogue pms/W2_SCALE + b2 + y."""
    with (
        tc.tile_pool(name="bc2", bufs=2) as bc2p,
        tc.tile_pool(name="h0p", bufs=10) as h0p,
        tc.tile_pool(name="h0tmp", bufs=2) as h0tmpp,
        tc.tile_pool(name="g1p", bufs=17) as g1p,
        tc.tile_pool(name="w2p", bufs=3) as w2p,
        tc.tile_pool(name="ep", bufs=3) as epp,
        tc.tile_pool(name="yo", bufs=3) as yop,
        tc.tile_pool(name="ps_f", bufs=8, space="PSUM") as ps_f,
    ):
        for tci in range(4):
            t0 = tci * NC
            qb2 = bc2p.tile([P, NC], F32, tag="qb2", name="qb2")
            rb2 = bc2p.tile([P, NC], F32, tag="rb2", name="rb2")
            nc.sync.dma_start(out=qb2, in_=_bcast_ap(dr.r2d, t0, NC, P))
            nc.sync.dma_start(out=rb2, in_=_bcast_ap(dr.r2d, NT + t0, NC, P))
            h0c = []
            for dt in range(8):
                ht = h0tmpp.tile([P, NC], F32, tag="h0tmp", name="h0tmp")
                nc.vector.tensor_mul(ht[:, :], y[dt][:, t0:t0 + NC], rb2[:, :])
                h0 = h0p.tile([P, NC], BF16, tag="h0c", name="h0c")
                nc.vector.tensor_sub(h0[:, :], ht[:, :], qb2[:, :])
                h0c.append(h0)
            g1 = [g1p.tile([P, 2, NC], F8E4, tag="g1", name="g1")
                  for _ in range(16)]
            for j in range(32):
                pm = ps_f.tile([P, NC], F32, tag="ps_f", name="ps_f")
                for kc in range(8):
                    nc.tensor.matmul(
                        pm[:, :], w1_sb[kc][:, j * P:(j + 1) * P],
                        h0c[kc][:, :], start=(kc == 0), stop=(kc == 7),
                    )
                nc.scalar.activation(
                    out=g1[j // 2][:, j % 2, :], in_=pm[:, :], func=AF.Gelu,
                    bias=cst.b1_sb[:, j:j + 1],
                )
            pms = [ps_f.tile([P, NC], F32, tag="ps_f", name="ps_f")
                   for _ in range(8)]
            for kj in range(16):
                w2t = w2p.tile([P, 2, 1024], F8E4, tag="w2t", name="w2t")
                nc.sync.dma_start(
                    out=w2t, in_=io.w2f[:, kj * 2048:(kj + 1) * 2048]
                )
                for k in range(8):
                    nc.tensor.matmul(
                        pms[k][:, :], w2t[:, :, k * P:(k + 1) * P],
                        g1[kj][:, :, :], start=(kj == 0), stop=(kj == 15),
                        perf_mode=DR,
                    )
            for k in range(8):
                ep = epp.tile([P, NC], F32, tag="ep", name="ep")
                nc.scalar.activation(
                    out=ep[:, :], in_=pms[k][:, :], func=AF.Identity,
                    bias=cst.b2_sb[:, k:k + 1], scale=1.0 / W2_SCALE,
                )
                yo = yop.tile([P, NC], F32, tag="yo", name="yo")
                nc.vector.tensor_add(yo[:, :], ep[:, :], y[k][:, t0:t0 + NC])
                nc.sync.dma_start(
                    out=io.yout[k * P:(k + 1) * P, t0:t0 + NC], in_=yo
                )


class _NS:
    def __init__(self, **kw):
        self.__dict__.update(kw)


def build_nc(stage=6):
    nc = bacc.Bacc(None, target_bir_lowering=False, debug=False)

    io = _NS(
        xt=nc.dram_tensor("xt", [D, NT], F32R, kind="ExternalInput"),
        mktp=nc.dram_tensor("mktp", [8, P, S], F32R, kind="ExternalInput"),
        cb=nc.dram_tensor("cb", [P, 64], F32, kind="ExternalInput"),
        mva=nc.dram_tensor("mva", [64, P, 65], F32, kind="ExternalInput"),
        wot=nc.dram_tensor("wot", [D, D], BF16, kind="ExternalInput"),
        w1=nc.dram_tensor("w1", [D, DFF], BF16, kind="ExternalInput"),
        b1p=nc.dram_tensor("b1p", [P, 32], F32, kind="ExternalInput"),
        w2f=nc.dram_tensor("w2f", [P, 32 * 1024], F8E4, kind="ExternalInput"),
        b2p=nc.dram_tensor("b2p", [P, 8], F32, kind="ExternalInput"),
        onesf=nc.dram_tensor("onesf", [P, 1], F32, kind="ExternalInput"),
        yout=nc.dram_tensor("yout", [D, NT], F32, kind="ExternalOutput"),
    )
    debug = os.environ.get("KERNEL_DEBUG", "0") == "1"
    if debug:
        io.dbgf = nc.dram_tensor("dbgf", [2072, NT], F32R, kind="ExternalOutput")
        io.dbg16 = nc.dram_tensor("dbg16", [1024, NT], BF16, kind="ExternalOutput")
    else:
        io.dbgf = None
        io.dbg16 = None
    groups = [[0, 1], [2, 3], [4, 5], [6, 7]]

    with tile.TileContext(nc) as tc:
        with (
            tc.tile_pool(name="dram", bufs=1, space="DRAM") as dram,
            tc.tile_pool(name="consts", bufs=1) as consts,
        ):
            dr = _NS(
                s1d=dram.tile([2, NT], F32, tag="s1d", name="s1d"),
                r1d=dram.tile([2, NT], F32, tag="r1d", name="r1d"),
                s2d=dram.tile([2, NT], F32, tag="s2d", name="s2d"),
                r2d=dram.tile([2, NT], F32, tag="r2d", name="r2d"),
                ogd=dram.tile([D, NT], BF16, tag="ogd", name="ogd"),
                dinv_raw=dram.tile([H, NT], F32, tag="dinv_raw", name="dinv_raw"),
                dinv_inv=dram.tile([H, NT], F32, tag="dinv_inv", name="dinv_inv"),
                zin=dram.tile([H, P, 4], F32, tag="zin", name="zin"),
                zout=dram.tile([H, P, 4], F32, tag="zout", name="zout"),
            )

            cst = _NS(
                onesr=consts.tile([P, 1], F32R, tag="onesr", name="onesr"),
                cb_sb=consts.tile([P, 64], F32, tag="cb_sb", name="cb_sb"),
                b1_sb=consts.tile([P, 32], F32, tag="b1_sb", name="b1_sb"),
                b2_sb=consts.tile([P, 8], F32, tag="b2_sb", name="b2_sb"),
            )
            nc.sync.dma_start(out=cst.onesr, in_=io.onesf[:, :].bitcast(F32R))
            nc.sync.dma_start(out=cst.cb_sb, in_=io.cb[:, :])
            nc.sync.dma_start(out=cst.b1_sb, in_=io.b1p[:, :])
            nc.sync.dma_start(out=cst.b2_sb, in_=io.b2p[:, :])

            stagev = stage

            with (
                tc.tile_pool(name="xg", bufs=8) as xg_pool,
                tc.tile_pool(name="og", bufs=8) as og_pool,
                tc.tile_pool(name="mktp_p", bufs=8) as mktp_pool,
            ):
                xg = [xg_pool.tile([P, NT], F32R, tag="xg", name="xg")
                      for _ in range(8)]
                og = [og_pool.tile([P, NT], BF16, tag="og", name="og")
                      for _ in range(8)]
                io.mktp_sb = [
                    mktp_pool.tile([P, S], F32R, tag="mktp", name="mktp")
                    for _ in range(8)
                ]
                for j in range(8):
                    nc.sync.dma_start(out=io.mktp_sb[j], in_=io.mktp[j, :, :])
                if stagev >= 1:
                    _emit_ln1(nc, tc, io, dr, cst, xg)
                    if debug:
                        for dt in range(8):
                            nc.sync.dma_start(
                                out=io.dbgf[dt * P:(dt + 1) * P, :], in_=xg[dt]
                            )
                if stagev >= 2:
                    _emit_attention(nc, tc, io, dr, cst, xg, og, groups,
                                    stagev)

            with (
                tc.tile_pool(name="w1p", bufs=8) as w1p,
                tc.tile_pool(name="wotp", bufs=8) as wotp,
                tc.tile_pool(name="yp", bufs=8) as yp,
            ):
                w1_sb = [w1p.tile([P, DFF], BF16, tag="w1_sb", name="w1_sb")
                         for _ in range(8)]
                wot_sb = [wotp.tile([P, D], BF16, tag="wot_sb", name="wot_sb")
                          for _ in range(8)]
                for kc in range(8):
                    nc.sync.dma_start(
                        out=wot_sb[kc], in_=io.wot[kc * P:(kc + 1) * P, :]
                    )
                    nc.sync.dma_start(
                        out=w1_sb[kc], in_=io.w1[kc * P:(kc + 1) * P, :]
                    )
                y = [yp.tile([P, NT], F32R, tag="y", name="y")
                     for _ in range(8)]
                if stagev >= 4:
                    _emit_conv(nc, tc, io, dr, cst, y, wot_sb)
                    if debug:
                        for dt in range(8):
                            nc.sync.dma_start(
                                out=io.dbgf[1024 + dt * P:
                                            1024 + (dt + 1) * P, :],
                                in_=y[dt],
                            )
                if stagev >= 6:
                    _emit_ffn(nc, tc, io, dr, cst, y, w1_sb)

    nc.finalize()
    return nc


def _prep_inputs(F_in, Mk, Mv, ln_g, ln_b, Wo, ln2_g, ln2_b, W1, b1, W2, b2):
    f = np.asarray(F_in, np.float32)
    Mk = np.asarray(Mk, np.float32)
    Mv = np.asarray(Mv, np.float32)
    ln_g = np.asarray(ln_g, np.float32)
    ln_b = np.asarray(ln_b, np.float32)
    ln2_g = np.asarray(ln2_g, np.float32)
    ln2_b = np.asarray(ln2_b, np.float32)
    W1 = np.asarray(W1, np.float32)
    W2 = np.asarray(W2, np.float32)
    b1 = np.asarray(b1, np.float32)
    b2 = np.asarray(b2, np.float32)
    Wo = np.asarray(Wo, np.float32)

    # Mk with ln_g folded, transposed per head, packed two heads per tile
    gmk = Mk * ln_g.reshape(H, 1, DH)            # (H, S, DH)
    mktp = np.empty((8, P, S), np.float32)
    for j in range(8):
        mktp[j, 0:64] = gmk[2 * j].T
        mktp[j, 64:128] = gmk[2 * j + 1].T
    # per-slot exp bias cb[h,s] = Mk[h,s,:] @ ln_b[h]
    cbf = np.einsum("hsd,hd->hs", Mk, ln_b.reshape(H, DH))   # (H, S)
    cb = np.zeros((P, 64), np.float32)
    for h in range(H):
        for st in range(4):
            cb[:, h * 4 + st] = cbf[h, st * P:(st + 1) * P]
    # Mv + ones column, per (head, slot-chunk)
    mva = np.empty((64, P, 65), np.float32)
    for h in range(H):
        for st in range(4):
            mva[h * 4 + st, :, 0:64] = Mv[h, st * P:(st + 1) * P, :]
            mva[h * 4 + st, :, 64] = 1.0
    wot = np.ascontiguousarray(Wo.T).astype(ml_dtypes.bfloat16)
    w1f = (ln2_g.reshape(D, 1) * W1).astype(ml_dtypes.bfloat16)
    b1f = np.ascontiguousarray(
        (b1 + ln2_b @ W1).reshape(32, P).T)          # [128, 32]
    # W2 quantized to TRN e4m3 in DoubleRow [pi, po, d] layout
    w2r = (W2.reshape(32, P, D) * W2_SCALE).transpose(1, 0, 2)   # [128,32,1024]
    w2f = np.ascontiguousarray(w2r.reshape(P, 32 * 1024)).astype(
        ml_dtypes.float8_e4m3)
    b2p = np.ascontiguousarray(b2.reshape(8, P).T)   # [128, 8]
    onesf = np.ones((P, 1), np.float32)

    in_maps = []
    for core in range(8):
        b, g = core // 2, core % 2
        xt = np.ascontiguousarray(f[b].T[:, g * NT:(g + 1) * NT])
        in_maps.append({
            "xt": xt, "mktp": mktp, "cb": cb, "mva": mva,
            "wot": wot, "w1": w1f, "b1p": b1f, "w2f": w2f, "b2p": b2p,
            "onesf": onesf,
        })
    return in_maps


def run_on_hw(in_maps, **kwargs):
    stage = int(os.environ.get("KERNEL_STAGE", "6"))
    key = (stage, os.environ.get("KERNEL_HEADS"), os.environ.get("KERNEL_DEBUG"))
    if key not in _CACHED:
        _CACHED[key] = build_nc(stage)
    return run_bass_kernel_spmd(_CACHED[key], in_maps, list(range(8)), **kwargs)


def kernel(**inputs) -> np.ndarray:
    in_maps = _prep_inputs(**inputs)
    res = run_on_hw(in_maps)
    outs = [res.results[i]["yout"] for i in range(8)]
    full = np.empty((B, N, D), np.float32)
    for b in range(B):
        yt = np.concatenate([outs[2 * b], outs[2 * b + 1]], axis=1)  # (D, N)
        full[b] = yt.T
    return full
